# revision 1
# baseline (speedup 1.0000x reference)
"""Trainium2 Bass kernel v2 for EnhancedLocalComplexAttention.

Reference semantics (complex windowed attention):
  x = x_re + i*x_im                     [b=2, n=4096, dim=512]
  q = x @ wq.T ; k = x @ wk.T ; v = x @ wv.T          (complex, 512x512)
  per head (8 heads x 64) and non-overlapping 128-token window:
    dots = real(q . conj(k)) * scale + rel_bias[j-i+128]
    attn = softmax(dots); out = attn @ v  (attn real)
  y = out @ wo.T  (complex); return stack([y.re, y.im])  [2, b, n, dim]

v2 design vs v1:
  - All matmuls bf16 (PSUM f32). Host pre-casts x and weights to bf16:
    halves DMA bytes and SBUF, same PE rate as f32r.
  - Stacked complex weights (host-side): for q/k/v, head h's PSUM tile
    accumulates [*_re(h); *_im(h)] on 128 partitions directly:
      A[:, (h, c, d)] = [wre | wim] columns   (multiplies x_re)
      B[:, (h, c, d)] = [-wim | wre] columns  (multiplies x_im)
    -> one [128,512] copy per head, no partition-shifted copies, and the
    negated-imag weight variants are free (folded into B).
  - v / o projections use feature order f' = (h, comp, d) so attention
    output PSUM tiles copy straight into the o-proj contraction layout.
  - Attention runs per (head, 4 windows) group on [128, 4*128] tiles:
    rel-bias preloaded into PSUM by the Act engine, dots matmuls
    accumulate on top (start=False), one big exp (Act), grouped row-sum
    (DVE), 4 transposes + 4 pv matmuls, one [128,512] copy back.
  - Batch interleave: attn(b0) groups braided with qkv-proj(b1) matmuls,
    attn(b1) braided with o-proj(b0), so PE never idles on softmax.
"""

import numpy as np
import ml_dtypes

P = 128         # SBUF partitions / window size
DIM = 512
NKT = DIM // P  # 4 k-tiles
TOK = 512       # tokens per core per batch
NW = TOK // P   # 4 windows per chunk
NB = 2          # batches
NH = 8          # heads
HD = 64         # head dim
N_CORES = 8
N = 4096
SCALE = HD ** (-0.5)

QKV_NAMES = ["wqA", "wqB", "wkA", "wkB", "wvA", "wvB"]
O_NAMES = ["woA", "woB"]

_COMPILED = {}
LAST_RESULT = None


def _build_program(loop_n=None):
    import concourse.bacc as bacc
    import concourse.mybir as mybir
    import concourse.tile as tile
    from contextlib import ExitStack

    f32 = mybir.dt.float32
    f32r = mybir.dt.float32r
    bf16 = mybir.dt.bfloat16

    nc = bacc.Bacc(
        "TRN2",
        target_bir_lowering=False,
        debug=False,
        enable_asserts=False,
        num_devices=N_CORES,
    )

    # All inputs pre-swizzled on host to partition-major layouts so every
    # DMA line is >=2KB contiguous. wq in 4 chunks (2 heads each), wk/wv in
    # 2 chunks: fine-grained arrivals match phase-1 consumption order.
    ins = {}
    for name in ["xT_re", "xT_im"]:
        ins[name] = nc.dram_tensor(name, [NB, NKT, P, TOK], bf16, kind="ExternalInput").ap()
    for name in ["wqA0", "wqB0"]:   # heads 0-1
        ins[name] = nc.dram_tensor(name, [P, NKT, 2 * P], bf16, kind="ExternalInput").ap()
    for name in ["wqA1", "wqB1"]:   # heads 2-7
        ins[name] = nc.dram_tensor(name, [P, NKT, 6 * P], bf16, kind="ExternalInput").ap()
    for name in ["wkA", "wkB", "wvA", "wvB"]:
        ins[name] = nc.dram_tensor(name, [P, NKT, NH * P], bf16, kind="ExternalInput").ap()
    for name in O_NAMES:
        ins[name] = nc.dram_tensor(name, [P, NH, DIM], f32r, kind="ExternalInput").ap()
    ins["bias4"] = nc.dram_tensor("bias4", [P, NW * P], bf16, kind="ExternalInput").ap()
    ins["ident"] = nc.dram_tensor("ident", [P, P], bf16, kind="ExternalInput").ap()
    outs = {
        "y_re": nc.dram_tensor("y_re", [NB, TOK, DIM], f32, kind="ExternalOutput").ap(),
        "y_im": nc.dram_tensor("y_im", [NB, TOK, DIM], f32, kind="ExternalOutput").ap(),
    }

    with tile.TileContext(nc) as tc, ExitStack() as ctx:
        wpool = ctx.enter_context(tc.tile_pool(name="wpool", bufs=1))
        cpool = ctx.enter_context(tc.tile_pool(name="cpool", bufs=1))
        xpool = ctx.enter_context(tc.tile_pool(name="xpool", bufs=2))
        qkpool = ctx.enter_context(tc.tile_pool(name="qkpool", bufs=2))
        vpool = ctx.enter_context(tc.tile_pool(name="vpool", bufs=2))
        aopool = ctx.enter_context(tc.tile_pool(name="aopool", bufs=2))
        ypool = ctx.enter_context(tc.tile_pool(name="ypool", bufs=4))
        sc = ctx.enter_context(tc.tile_pool(name="sc", bufs=3))
        pp_proj = ctx.enter_context(tc.tile_pool(name="pp_proj", bufs=3, space="PSUM"))
        pp_dots = ctx.enter_context(tc.tile_pool(name="pp_dots", bufs=3, space="PSUM"))
        pp_tp = ctx.enter_context(tc.tile_pool(name="pp_tp", bufs=2, space="PSUM"))

        # --- resident constants -------------------------------------------
        # DMA issue order == consumption order: x(b0), wq chunks, wk, wv,
        # x(b1), bias/ident, wo. A-halves on the scalar queue, B-halves on
        # gpsimd, x on sync.
        w_sb = {}
        for name in QKV_NAMES:
            w_sb[name] = wpool.tile([P, NKT, NH * P], bf16, name=f"sb_{name}")
        for name in O_NAMES:
            w_sb[name] = wpool.tile([P, NH, DIM], f32r, name=f"sb_{name}")

        x_sb_all = []
        for b in range(NB):
            xre = xpool.tile([P, NKT, TOK], bf16, tag="xre", name=f"xre{b}")
            xim = xpool.tile([P, NKT, TOK], bf16, tag="xim", name=f"xim{b}")
            x_sb_all.append((xre, xim))
        # b0 x on sync; wq split [heads 0-1 | heads 2-7] so the first unit's
        # deps are ~1.5MB; wk/wv as whole-tensor loads (HWDGE per-DMA cost
        # ~620ns makes fine chunks counterproductive).
        nc.sync.dma_start(out=x_sb_all[0][0],
                          in_=ins["xT_re"][0].rearrange("kt p t -> p kt t"))
        nc.sync.dma_start(out=x_sb_all[0][1],
                          in_=ins["xT_im"][0].rearrange("kt p t -> p kt t"))
        nc.scalar.dma_start(out=w_sb["wqA"][:, :, 0:2 * P], in_=ins["wqA0"])
        nc.gpsimd.dma_start(out=w_sb["wqB"][:, :, 0:2 * P], in_=ins["wqB0"])
        nc.scalar.dma_start(out=w_sb["wqA"][:, :, 2 * P:], in_=ins["wqA1"])
        nc.gpsimd.dma_start(out=w_sb["wqB"][:, :, 2 * P:], in_=ins["wqB1"])
        for nm in ["wk", "wv"]:
            nc.scalar.dma_start(out=w_sb[nm + "A"], in_=ins[nm + "A"])
            nc.gpsimd.dma_start(out=w_sb[nm + "B"], in_=ins[nm + "B"])
        bias_sb = cpool.tile([P, NW * P], bf16, name="bias_sb")
        nc.scalar.dma_start(out=bias_sb, in_=ins["bias4"])
        id_sb = cpool.tile([P, P], bf16, name="id_sb")
        nc.gpsimd.dma_start(out=id_sb, in_=ins["ident"])
        nc.scalar.dma_start(out=w_sb["woA"], in_=ins["woA"])
        nc.gpsimd.dma_start(out=w_sb["woB"], in_=ins["woB"])
        # x(b1) at the END of the weight queues (FIFO): can't starve the
        # phase-1 weight stream, still lands long before phase 2 needs it.
        nc.scalar.dma_start(out=x_sb_all[1][0], in_=ins["xT_re"][1].rearrange("kt p t -> p kt t"))
        nc.gpsimd.dma_start(out=x_sb_all[1][1], in_=ins["xT_im"][1].rearrange("kt p t -> p kt t"))

        def cp(eng, out, in_):
            if hasattr(eng, "tensor_copy"):
                eng.tensor_copy(out=out, in_=in_)
            else:
                eng.copy(out=out, in_=in_)

        def body():
            x_sb = x_sb_all
            st = {}  # per-batch tiles

            def proj_qkv_units(b):
                """24 psum-producing units (8 q-heads, 8 k-heads, 8 v-halves)
                of 8 matmuls + 1 copy each."""
                xre, xim = x_sb[b]
                qcat = qkpool.tile([P, NH, TOK], bf16, tag="qcat", name=f"qcat{b}")
                kcat = qkpool.tile([P, NH, TOK], bf16, tag="kcat", name=f"kcat{b}")
                vcat = vpool.tile([P, NW, 2, DIM], bf16, tag="vcat", name=f"vcat{b}")
                st[b] = dict(qcat=qcat, kcat=kcat, vcat=vcat)

                def qk_unit(dst, wA, wB, h, ceng):
                    def run():
                        ps = pp_proj.tile([P, TOK], f32, tag="proj",
                                          name=f"ps_{wA}{b}_{h}")
                        mm = 0
                        for kt in range(NKT):   # kt-major: matches DMA arrival
                            for wname, xs in ((wA, xre), (wB, xim)):
                                nc.tensor.matmul(
                                    ps,
                                    w_sb[wname][:, kt, h * P:(h + 1) * P],
                                    xs[:, kt, :],
                                    start=(mm == 0), stop=(mm == 7),
                                )
                                mm += 1
                        cp(ceng, dst[:, h, :], ps)
                    return run

                def v_unit(it, fh, ceng):
                    def run():
                        ps = pp_proj.tile([P, DIM], f32, tag="proj",
                                          name=f"ps_v{b}_{it}_{fh}")
                        mm = 0
                        for wname, xs in (("wvA", xre), ("wvB", xim)):
                            for kt in range(NKT):
                                nc.tensor.matmul(
                                    ps,
                                    xs[:, kt, it * P:(it + 1) * P],
                                    w_sb[wname][:, kt, fh * DIM:(fh + 1) * DIM],
                                    start=(mm == 0), stop=(mm == 7),
                                )
                                mm += 1
                        cp(ceng, vcat[:, it, fh, :], ps)
                    return run

                units = []
                for h in range(NH):
                    units.append(qk_unit(qcat, "wqA", "wqB", h,
                                         nc.scalar if h % 2 else nc.vector))
                for h in range(NH):
                    units.append(qk_unit(kcat, "wkA", "wkB", h,
                                         nc.vector if h % 2 else nc.scalar))
                for it in range(NW):
                    for fh in range(2):
                        units.append(v_unit(it, fh,
                                            nc.scalar if fh else nc.vector))
                return units

            def attn_units(b):
                """Per-head attention groups as pipeline stages."""
                ao = aopool.tile([P, NH, TOK], f32r, tag="ao", name=f"ao{b}")
                st[b]["ao"] = ao
                stash = {}

                def front(h):
                    qcat, kcat = st[b]["qcat"], st[b]["kcat"]
                    pd = pp_dots.tile([P, NW, P], f32, tag="dots", name=f"pd{b}_{h}")
                    nc.scalar.copy(
                        out=pd, in_=bias_sb.rearrange("p (w j) -> p w j", w=NW))
                    for w in range(NW):
                        nc.tensor.matmul(
                            pd[:, w, :],
                            qcat[:, h, w * P:(w + 1) * P],
                            kcat[:, h, w * P:(w + 1) * P],
                            start=False, stop=True, skip_group_check=True,
                        )
                    e = sc.tile([P, NW, P], bf16, tag="e", name=f"e{b}_{h}", bufs=3)
                    nc.scalar.activation(out=e, in_=pd,
                                         func=mybir.ActivationFunctionType.Exp)
                    s = sc.tile([P, NW], f32, tag="s", name=f"s{b}_{h}", bufs=4)
                    nc.vector.tensor_reduce(out=s, in_=e, axis=mybir.AxisListType.X,
                                            op=mybir.AluOpType.add)
                    stash[h] = (e, s)

                def mid(h):
                    e, s = stash[h]
                    rcp = sc.tile([P, NW], f32, tag="r", name=f"r{b}_{h}", bufs=4)
                    nc.vector.reciprocal(rcp, s)
                    a = sc.tile([P, NW, P], bf16, tag="a", name=f"a{b}_{h}", bufs=3)
                    for w in range(NW):
                        nc.vector.tensor_scalar_mul(a[:, w, :], e[:, w, :],
                                                    rcp[:, w:w + 1])
                    stash[h] = a

                def back_tp(h):
                    a = stash[h]
                    pt = pp_tp.tile([P, NW, P], bf16, tag="tp", name=f"pt{b}_{h}")
                    for w in range(NW):
                        nc.tensor.transpose(pt[:, w, :], a[:, w, :], id_sb)
                    at = sc.tile([P, NW, P], bf16, tag="at", name=f"at{b}_{h}", bufs=3)
                    nc.vector.tensor_copy(out=at, in_=pt)
                    stash[h] = at

                def back_pv(h):
                    at = stash.pop(h)
                    vcat = st[b]["vcat"]
                    pv = pp_dots.tile([P, NW, P], f32, tag="dots", name=f"pv{b}_{h}")
                    for w in range(NW):
                        nc.tensor.matmul(
                            pv[:, w, :],
                            vcat[:, w, h // 4, (h % 4) * P:(h % 4 + 1) * P],
                            at[:, w, :],
                            start=True, stop=True,
                        )
                    nc.scalar.copy(out=st[b]["ao"][:, h, :],
                                          in_=pv.rearrange("p w j -> p (w j)"))

                return front, mid, back_tp, back_pv

            def oproj_units(b):
                """8 units: (it, comp): 8 matmuls + copy + store."""
                ao = st[b]["ao"]

                def unit(it, oname, wname, ceng, deng):
                    def run():
                        ps = pp_proj.tile([P, DIM], f32, tag="proj",
                                          name=f"ps_{oname}{b}{it}")
                        for h in range(NH):
                            nc.tensor.matmul(
                                ps,
                                ao[:, h, it * P:(it + 1) * P],
                                w_sb[wname][:, h, :],
                                start=(h == 0), stop=(h == 7),
                            )
                        ys = ypool.tile([P, DIM], f32, tag="y", name=f"ys_{oname}{b}{it}")
                        # copy split across both engines to halve drain latency
                        cp(nc.scalar, ys[:, 0:DIM // 2], ps[:, 0:DIM // 2])
                        cp(nc.vector, ys[:, DIM // 2:], ps[:, DIM // 2:])
                        deng.dma_start(
                            out=outs[oname][b, it * P:(it + 1) * P, :], in_=ys)
                    return run

                qrot = [nc.gpsimd, nc.sync, nc.scalar]
                units = []
                for it in range(NW):
                    units.append(unit(it, "y_re", "woA", None,
                                      qrot[(2 * it) % 3]))
                    units.append(unit(it, "y_im", "woB", None,
                                      qrot[(2 * it + 1) % 3]))
                return units

            # --------------- schedule -----------------------------------
            # phase 1: qkv proj b0 (24 units)
            for u in proj_qkv_units(0):
                u()
            # phase 2: attn b0 braided with qkv proj b1 (3 units per head)
            f0, m0, t0, p0 = attn_units(0)
            pb1 = proj_qkv_units(1)
            for h in range(NH + 2):
                if h < NH:
                    f0(h)
                    for u in pb1[3 * h:3 * (h + 1)]:
                        u()
                if 1 <= h < NH + 1:
                    m0(h - 1)
                    t0(h - 1)
                if h >= 2:
                    p0(h - 2)
            # phase 3: attn b1 braided with oproj b0 (1 unit per head)
            f1, m1, t1, p1 = attn_units(1)
            ob0 = oproj_units(0)
            for h in range(NH + 2):
                if h < NH:
                    f1(h)
                    ob0[h]()
                if 1 <= h < NH + 1:
                    m1(h - 1)
                    t1(h - 1)
                if h >= 2:
                    p1(h - 2)
            # phase 4: oproj b1
            for u in oproj_units(1):
                u()

        if loop_n:
            with tc.For_i(0, loop_n):
                body()
        else:
            body()

    nc.compile()
    return nc


def get_compiled(loop_n=None):
    key = loop_n
    if key not in _COMPILED:
        _COMPILED[key] = _build_program(loop_n)
    return _COMPILED[key]


def make_in_maps(x_re, x_im, wq_re, wq_im, wk_re, wk_im, wv_re, wv_im,
                 wo_re, wo_im, rel_bias):
    """Host-side prep: bf16 cast, stacked complex weights, token sharding."""
    f32 = np.float32
    bf16 = ml_dtypes.bfloat16

    def stack_qk(wre, wim, scale=1.0):
        # [dim(k), NH*P(f')], f' = h*128 + comp*64 + d; A mult x_re, B mult x_im
        wre = np.asarray(wre, f32).T * scale   # [k, f] natural f = h*64+d
        wim = np.asarray(wim, f32).T * scale
        A = np.empty((DIM, NH * P), f32)
        B = np.empty((DIM, NH * P), f32)
        for h in range(NH):
            A[:, h * P:h * P + HD] = wre[:, h * HD:(h + 1) * HD]
            A[:, h * P + HD:(h + 1) * P] = wim[:, h * HD:(h + 1) * HD]
            B[:, h * P:h * P + HD] = -wim[:, h * HD:(h + 1) * HD]
            B[:, h * P + HD:(h + 1) * P] = wre[:, h * HD:(h + 1) * HD]
        return A.astype(bf16), B.astype(bf16)

    def stack_o(wo_re, wo_im):
        # rows f' = (h, comp, d); A -> y_re, B -> y_im
        wor = np.asarray(wo_re, f32)  # [j, f]
        woi = np.asarray(wo_im, f32)
        A = np.empty((NH * P, DIM), f32)
        B = np.empty((NH * P, DIM), f32)
        for h in range(NH):
            A[h * P:h * P + HD, :] = wor[:, h * HD:(h + 1) * HD].T
            A[h * P + HD:(h + 1) * P, :] = -woi[:, h * HD:(h + 1) * HD].T
            B[h * P:h * P + HD, :] = woi[:, h * HD:(h + 1) * HD].T
            B[h * P + HD:(h + 1) * P, :] = wor[:, h * HD:(h + 1) * HD].T
        return A, B

    wqA, wqB = map(lambda a: a.astype(bf16), stack_qk(wq_re, wq_im, SCALE))
    wkA, wkB = map(lambda a: a.astype(bf16), stack_qk(wk_re, wk_im))
    wvA, wvB = map(lambda a: a.astype(bf16), stack_qk(wv_re, wv_im))
    woA, woB = stack_o(wo_re, wo_im)

    def swz_chunks(A, nch):
        # [DIM(k), NJ] -> [nch, P, NKT, NJ//nch] partition-major chunks
        nj = A.shape[1] // nch
        out = np.empty((nch, P, NKT, nj), A.dtype)
        for c in range(nch):
            out[c] = A[:, c * nj:(c + 1) * nj].reshape(NKT, P, nj).transpose(1, 0, 2)
        return np.ascontiguousarray(out)

    def swz_o(A):
        # [NH*P(f'), DIM] -> [P, NH, DIM] partition-major
        return np.ascontiguousarray(A.reshape(NH, P, DIM).transpose(1, 0, 2))

    idx = np.arange(P)[None, :] - np.arange(P)[:, None] + P
    bias_mat = np.asarray(rel_bias, f32)[idx]
    wqAs, wqBs = swz_chunks(wqA, 1)[0], swz_chunks(wqB, 1)[0]
    shared = {
        "wqA0": np.ascontiguousarray(wqAs[:, :, 0:2 * P]),
        "wqB0": np.ascontiguousarray(wqBs[:, :, 0:2 * P]),
        "wqA1": np.ascontiguousarray(wqAs[:, :, 2 * P:]),
        "wqB1": np.ascontiguousarray(wqBs[:, :, 2 * P:]),
        "wkA": swz_chunks(wkA, 1)[0], "wkB": swz_chunks(wkB, 1)[0],
        "wvA": swz_chunks(wvA, 1)[0], "wvB": swz_chunks(wvB, 1)[0],
        "woA": swz_o(woA), "woB": swz_o(woB),
        "bias4": np.ascontiguousarray(np.tile(bias_mat, (1, NW))).astype(bf16),
        "ident": np.eye(P, dtype=bf16),
    }

    x_re = np.asarray(x_re, f32).astype(bf16)
    x_im = np.asarray(x_im, f32).astype(bf16)
    in_maps = []
    for c in range(N_CORES):
        sl = slice(c * TOK, (c + 1) * TOK)
        m = dict(shared)
        # [b, tok, dim] -> [b, NKT, P, TOK] with row k = kt*P + p
        m["xT_re"] = np.ascontiguousarray(
            x_re[:, sl, :].transpose(0, 2, 1).reshape(NB, NKT, P, TOK))
        m["xT_im"] = np.ascontiguousarray(
            x_im[:, sl, :].transpose(0, 2, 1).reshape(NB, NKT, P, TOK))
        in_maps.append(m)
    return in_maps


def assemble_output(results):
    out = np.empty((2, NB, N, DIM), np.float32)
    for c in range(N_CORES):
        sl = slice(c * TOK, (c + 1) * TOK)
        out[0, :, sl, :] = results[c]["y_re"].astype(np.float32)
        out[1, :, sl, :] = results[c]["y_im"].astype(np.float32)
    return out


def kernel(**inputs):
    global LAST_RESULT
    import os
    from concourse.bass_utils import run_bass_kernel_spmd

    nc = get_compiled()
    in_maps = make_in_maps(**inputs)
    core_ids = list(range(N_CORES))
    try:
        res = run_bass_kernel_spmd(nc, in_maps, core_ids)
    except ModuleNotFoundError:
        os.environ["BASS_NEVER_TRACE"] = "1"
        res = run_bass_kernel_spmd(nc, in_maps, core_ids)
    LAST_RESULT = res
    return assemble_output(res.results)

